# revision 22
# baseline (speedup 1.0000x reference)
"""Trainium2 Bass kernel for nn_ConcatRelationModule (gnn_message_passing).

Strategy: data-parallel over the edge dimension E across 8 NeuronCores.
 - Sharding/prep (host, untimed): edges split contiguously, 32768 per core.
   The per-edge head rows fwd[gold_heads] are materialized during sharding
   (the "gathered for the head indices" option of the sharding hint) and the
   modifier rows bwd[e+1] are a contiguous slice; both ship pre-transposed
   as fp16 [128, 32768] per core. The hinge mask ships as
   m2[p, b, r] = routBias[r] - 1024*(gold_rels[e]==r)  (e = b*128+p), fp16.
 - Device kernel, software-pipelined over 32 groups of 1024 edges:
     PE:     fov/mov (4+4 matmuls, n=512), h2 (4 matmuls) one group behind,
             scores edge-major (8 matmuls [128e,64l]) three groups behind so
             the scalar engine has a full step of slack to produce tanh(h2).
     Scalar: three tanh activations per group (the throughput floor,
             1 elem/lane/cycle @ 1.2 GHz).
     Vector: hinge = TT add(scores, m2) -> reduce_max (best wrong+bias)
             and reduce_min (gold+bias-1024, exact), 3 ops per group.
     DMA:    split across the SP hwdge queue and gpsimd swdge queue; one
             queue (~224 GB/s) cannot feed the ~25 MB/core input stream.
   lerrs accumulate in SBUF [128, 256] and are transposed out per half.
"""
import sys

sys.path.insert(0, "/opt/trn_rl_repo")

import numpy as np

import concourse.bass as bass
import concourse.bacc as bacc
import concourse.mybir as mybir
import concourse.tile as tile
from concourse.bass_utils import run_bass_kernel_spmd
from concourse.masks import make_identity

F32 = mybir.dt.float32
F16 = mybir.dt.float16

N = 262144
L = 128
H = 128
H2 = 128
R = 64
E = N - 1
NCORES = 8
EPC = N // NCORES            # edges per core (the very last edge is padding)
NB = EPC // 128              # 256 blocks of 128 edges
GE = 1024                    # edges per pipeline group
NG = EPC // GE               # 32 groups
BPG = GE // 128              # 8 blocks per group
MOFF = -1024.0               # gold-mask offset (exact in fp16/fp32)


def build_kernel():
    nc = bacc.Bacc("TRN2", target_bir_lowering=False, debug=False)

    # xin rows 0:128 = fwd[heads].T, 128:256 = bwd[mods].T  (fp16)
    xin_d = nc.declare_dram_parameter("xin", [2 * L, EPC], F16, isOutput=False)
    # m2[p, b*64+r] = routBias[r] + MOFF*(rels[b*128+p]==r)  (fp32: the
    # routBias term must survive next to the 1024 offset; fp16 ulp(1024)=1)
    m2_d = nc.declare_dram_parameter("m2", [128, NB * R], F32, isOutput=False)
    # wpk cols (partition = contraction dim k): 0:128 WFOH[0:128],
    # 128:256 WFOH[128:256], 256:384 WFOM[0:128], 384:512 WFOM[128:256],
    # 512:640 rhid2[0:128], 640:768 rhid2[128:256], 768:832 rout
    wpk_d = nc.declare_dram_parameter("wpk", [128, 832], F16, isOutput=False)
    # b3 cols: 0 bcat[:128], 1 bcat[128:], 2 rhid2Bias
    b3_d = nc.declare_dram_parameter("b3", [128, 3], F32, isOutput=False)

    lerr_d = nc.declare_dram_parameter("lerr", [EPC], F32, isOutput=True)
    lerr_v = lerr_d[:].rearrange("(b p) -> b p", p=128)

    with tile.TileContext(nc) as tc:
        with (
            tc.tile_pool(name="const", bufs=1) as cp,
            tc.tile_pool(name="inp", bufs=3) as gp,
            tc.tile_pool(name="msk", bufs=4) as mp,
            tc.tile_pool(name="act", bufs=2) as ap,
            tc.tile_pool(name="work", bufs=2) as wp,
            tc.tile_pool(name="psA", bufs=1, space="PSUM") as pa,
            tc.tile_pool(name="psB", bufs=1, space="PSUM") as pb,
            tc.tile_pool(name="psS", bufs=2, space="PSUM") as ps,
        ):
            # ---- constants ----
            ident = cp.tile([128, 128], F32, tag="ident")
            make_identity(nc, ident[:])

            wall = cp.tile([128, 832], F16, tag="wall")
            b3 = cp.tile([128, 3], F32, tag="b3")
            wfoh_f = wall[:, 0:128]
            wfoh_b = wall[:, 128:256]
            wfom_f = wall[:, 256:384]
            wfom_b = wall[:, 384:512]
            rh2_a = wall[:, 512:640]
            rh2_b = wall[:, 640:768]
            rout = wall[:, 768:832]
            bias_h = b3[:, 0:1]
            bias_m = b3[:, 1:2]
            bias_2 = b3[:, 2:3]

            goldm_acc = cp.tile([128, NB], F32, tag="goldm_acc")
            wrong_acc = cp.tile([128, NB], F32, tag="wrong_acc")

            xf = [None] * NG
            xb = [None] * NG
            m2t = [None] * NG
            h1 = [None] * NG
            h1m = [None] * NG
            h2s = [None] * NG

            def dma_x(s):
                # split issue across the SP hwdge queue and the gpsimd swdge
                # queue: a single queue (~224 GB/s) cannot feed the pipeline
                xf[s] = gp.tile([128, GE], F16, tag="xf", name=f"xf{s}")
                nc.sync.dma_start(out=xf[s][:], in_=xin_d[0:128, s * GE:(s + 1) * GE])
                xb[s] = gp.tile([128, GE], F16, tag="xb", name=f"xb{s}")
                nc.gpsimd.dma_start(out=xb[s][:], in_=xin_d[128:256, s * GE:(s + 1) * GE])

            def dma_m2(s):
                m2t[s] = mp.tile([128, BPG * R], F32, tag="m2t", name=f"m2t{s}", bufs=6)
                eng = nc.gpsimd if s % 2 else nc.sync
                eng.dma_start(
                    out=m2t[s][:], in_=m2_d[:, s * BPG * R:(s + 1) * BPG * R])

            def dma_group(s):
                dma_x(s)
                dma_m2(s)

            def phase_a(s):
                # fov/mov for group s + the two tanh h1 activations
                fovp = pa.tile([128, GE], F32, tag="fovp")
                nc.tensor.matmul(out=fovp[:, 0:512], lhsT=wfoh_f[:],
                                 rhs=xf[s][:, 0:512], start=True, stop=False)
                nc.tensor.matmul(out=fovp[:, 512:1024], lhsT=wfoh_f[:],
                                 rhs=xf[s][:, 512:1024], start=True, stop=False)
                nc.tensor.matmul(out=fovp[:, 0:512], lhsT=wfoh_b[:],
                                 rhs=xb[s][:, 0:512], start=False, stop=True)
                nc.tensor.matmul(out=fovp[:, 512:1024], lhsT=wfoh_b[:],
                                 rhs=xb[s][:, 512:1024], start=False, stop=True)
                movp = pb.tile([128, GE], F32, tag="movp")
                nc.tensor.matmul(out=movp[:, 0:512], lhsT=wfom_f[:],
                                 rhs=xf[s][:, 0:512], start=True, stop=False)
                nc.tensor.matmul(out=movp[:, 512:1024], lhsT=wfom_f[:],
                                 rhs=xf[s][:, 512:1024], start=True, stop=False)
                nc.tensor.matmul(out=movp[:, 0:512], lhsT=wfom_b[:],
                                 rhs=xb[s][:, 0:512], start=False, stop=True)
                nc.tensor.matmul(out=movp[:, 512:1024], lhsT=wfom_b[:],
                                 rhs=xb[s][:, 512:1024], start=False, stop=True)
                h1[s] = ap.tile([128, GE], F16, tag="h1", name=f"h1_{s}")
                nc.scalar.activation(
                    out=h1[s][:], in_=fovp[:],
                    func=mybir.ActivationFunctionType.Tanh, bias=bias_h[:, 0:1])
                h1m[s] = ap.tile([128, GE], F16, tag="h1m", name=f"h1m_{s}")
                nc.scalar.activation(
                    out=h1m[s][:], in_=movp[:],
                    func=mybir.ActivationFunctionType.Tanh, bias=bias_m[:, 0:1])

            def phase_b(s):
                # h2 for group s (one group behind phase_a)
                h2pp = pa.tile([128, GE], F32, tag="h2pp")
                nc.tensor.matmul(out=h2pp[:, 0:512], lhsT=rh2_a[:],
                                 rhs=h1[s][:, 0:512], start=True, stop=False)
                nc.tensor.matmul(out=h2pp[:, 512:1024], lhsT=rh2_a[:],
                                 rhs=h1[s][:, 512:1024], start=True, stop=False)
                nc.tensor.matmul(out=h2pp[:, 0:512], lhsT=rh2_b[:],
                                 rhs=h1m[s][:, 0:512], start=False, stop=True)
                nc.tensor.matmul(out=h2pp[:, 512:1024], lhsT=rh2_b[:],
                                 rhs=h1m[s][:, 512:1024], start=False, stop=True)
                h2s[s] = ap.tile([128, GE], F16, tag="h2s", name=f"h2s_{s}", bufs=3)
                nc.scalar.activation(
                    out=h2s[s][:], in_=h2pp[:],
                    func=mybir.ActivationFunctionType.Tanh, bias=bias_2[:, 0:1])
                h1[s] = h1m[s] = None

            lerr_acc = cp.tile([128, NB], F32, tag="lerr_acc")

            def phase_c(s):
                # scores (edge-major) + full hinge for group s (3 behind)
                scp = ps.tile([128, BPG * R], F32, tag="scp")
                for j in range(BPG):
                    nc.tensor.matmul(
                        out=scp[:, j * R:(j + 1) * R],
                        lhsT=h2s[s][:, j * 128:(j + 1) * 128],
                        rhs=rout[:], start=True, stop=True)
                wm = wp.tile([128, BPG * R], F32, tag="wm")
                nc.vector.tensor_tensor(
                    out=wm[:], in0=scp[:], in1=m2t[s][:], op=mybir.AluOpType.add)
                wr8 = wrong_acc[:, s * BPG:(s + 1) * BPG]
                gm8 = goldm_acc[:, s * BPG:(s + 1) * BPG]
                nc.vector.reduce_max(
                    out=wr8, in_=wm[:].rearrange("p (j r) -> p j r", r=R),
                    axis=mybir.AxisListType.X)
                nc.vector.tensor_reduce(
                    out=gm8, in_=wm[:].rearrange("p (j r) -> p j r", r=R),
                    axis=mybir.AxisListType.X, op=mybir.AluOpType.min)
                # d = wrong - goldm + MOFF = wrong - gold;
                # lerr = (d > -1) ? d : 0    (gold = goldm - MOFF, exact)
                d8 = wp.tile([128, BPG], F32, tag="d8")
                nc.vector.tensor_tensor(
                    out=d8[:], in0=wr8, in1=gm8, op=mybir.AluOpType.subtract)
                d8b = wp.tile([128, BPG], F32, tag="d8b")
                nc.vector.tensor_scalar_add(out=d8b[:], in0=d8[:], scalar1=MOFF)
                nc.vector.scalar_tensor_tensor(
                    out=lerr_acc[:, s * BPG:(s + 1) * BPG],
                    in0=d8b[:], scalar=-1.0, in1=d8b[:],
                    op0=mybir.AluOpType.is_gt, op1=mybir.AluOpType.mult)
                h2s[s] = None
                m2t[s] = None
                xf[s] = xb[s] = None

            def emit_tail(half):
                # writeback for blocks [half*128, half*128+128)
                a = half * 128
                otp = ps.tile([128, BPG * R], F32, tag="scp", name=f"otp{half}")
                nc.tensor.transpose(
                    out=otp[:, 0:128], in_=lerr_acc[:, a:a + 128], identity=ident[:])
                osb = wp.tile([128, 128], F32, tag="osb", name=f"osb{half}")
                nc.scalar.copy(out=osb[:], in_=otp[:, 0:128])
                nc.sync.dma_start(out=lerr_v[a:a + 128, :], in_=osb[:])

            # ---- software-pipelined main loop ----
            # per step s: scores+hinge for s-3, fov/mov for s, h2 for s-1.
            # The 3-step trail gives the scalar engine a full step of slack
            # to finish tanh(h2) before the PE needs it for scores.
            # Startup order: group-0/1 x-data and the packed weights first so
            # the first matmul is not stuck behind a queue of small DMAs.
            # group-0 first halves, then weights, then the rest: the first
            # fov matmul needs only xf0[:, 0:512] + wall
            xf[0] = gp.tile([128, GE], F16, tag="xf", name="xf0")
            xb[0] = gp.tile([128, GE], F16, tag="xb", name="xb0")
            nc.sync.dma_start(out=xf[0][:, 0:512], in_=xin_d[0:128, 0:512])
            nc.gpsimd.dma_start(out=xb[0][:, 0:512], in_=xin_d[128:256, 0:512])
            nc.sync.dma_start(out=wall[:], in_=wpk_d[:])
            nc.gpsimd.dma_start(out=b3[:], in_=b3_d[:])
            nc.sync.dma_start(out=xf[0][:, 512:1024], in_=xin_d[0:128, 512:1024])
            nc.gpsimd.dma_start(out=xb[0][:, 512:1024], in_=xin_d[128:256, 512:1024])
            dma_x(1)
            dma_m2(0)
            dma_m2(1)
            for s in range(NG + 1):
                if s >= 3:
                    phase_c(s - 3)
                if s + 2 < NG:
                    dma_group(s + 2)
                if s < NG:
                    phase_a(s)
                if 1 <= s <= NG:
                    phase_b(s - 1)
                if s == NG:            # drain: C(NG-3) ran above; pull in C(NG-2)
                    phase_c(NG - 2)
                if s - 3 == NG // 2 - 1:
                    emit_tail(0)       # first half of blocks is complete
            phase_c(NG - 1)
            emit_tail(1)

    nc.compile()
    return nc


_NC_CACHE = {}


def _get_nc():
    if "nc" not in _NC_CACHE:
        _NC_CACHE["nc"] = build_kernel()
    return _NC_CACHE["nc"]


def make_weights(WFOH, WFOM, rcatBias, rhid2Layer, rhid2Bias, routLayer, routBias):
    WFOH = np.asarray(WFOH, np.float16)
    WFOM = np.asarray(WFOM, np.float16)
    rhid2 = np.asarray(rhid2Layer, np.float16)
    wpk = np.ascontiguousarray(np.hstack([
        WFOH[0:128], WFOH[128:256], WFOM[0:128], WFOM[128:256],
        rhid2[0:128], rhid2[128:256], np.asarray(routLayer, np.float16),
    ]))                                                 # [128, 832]
    bcat = np.asarray(rcatBias, np.float32).reshape(-1)
    b3 = np.ascontiguousarray(np.stack(
        [bcat[0:128], bcat[128:256],
         np.asarray(rhid2Bias, np.float32).reshape(-1)], axis=1))   # [128, 3]
    return dict(wpk=wpk, b3=b3)


def prepare_core_inputs(fwd, bwd, gold_heads, gold_rels, weights, routBias):
    """Shard edges contiguously; gather head rows / slice modifier rows.

    Core c owns edges [c*EPC, (c+1)*EPC); the single extra edge at the end
    (global index E) is padding with zero inputs.
    """
    fwd16 = np.asarray(fwd, dtype=np.float16)
    bwd16 = np.asarray(bwd, dtype=np.float16)
    heads = np.asarray(gold_heads, dtype=np.int64)
    rels = np.asarray(gold_rels, dtype=np.int64)
    rb32 = np.asarray(routBias, np.float32).reshape(-1)    # [R]

    in_maps = []
    for c in range(NCORES):
        lo, hi = c * EPC, (c + 1) * EPC
        if hi <= E:
            h_c = heads[lo:hi]
            fwd_rows = fwd16[h_c]                     # [EPC, L]
            bwd_rows = bwd16[lo + 1:hi + 1]           # contiguous modifiers
            r_c = rels[lo:hi]
        else:                                         # last core: pad 1 edge
            h_c = heads[lo:E]
            fwd_rows = np.zeros((EPC, L), np.float16)
            fwd_rows[:E - lo] = fwd16[h_c]
            bwd_rows = np.zeros((EPC, L), np.float16)
            bwd_rows[:E - lo] = bwd16[lo + 1:E + 1]
            r_c = np.zeros(EPC, np.int64)
            r_c[:E - lo] = rels[lo:E]

        xin = np.empty((2 * L, EPC), np.float16)
        xin[0:L] = fwd_rows.T
        xin[L:2 * L] = bwd_rows.T

        # m2[p, b, r] = routBias[r] + MOFF * (rels[b*128+p] == r)
        m2 = np.broadcast_to(rb32[None, None, :], (128, NB, R)).astype(np.float32).copy()
        idx = r_c.reshape(NB, 128).T                  # [128, NB]
        m2[np.arange(128)[:, None], np.arange(NB)[None, :], idx] += np.float32(MOFF)
        m2 = m2.reshape(128, NB * R)

        in_maps.append(dict(xin=xin, m2=m2, **weights))
    return in_maps


def assemble_output(results):
    lerr_full = np.empty(NCORES * EPC, dtype=np.float32)
    for c in range(NCORES):
        lerr_full[c * EPC:(c + 1) * EPC] = np.asarray(
            results[c]["lerr"], dtype=np.float32)
    return lerr_full[:E]


def kernel(fwd, bwd, gold_heads, gold_rels, WFOH, WFOM, rhidBias, rcatBias,
           rhid2Layer, rhid2Bias, routLayer, routBias):
    nc = _get_nc()
    weights = make_weights(WFOH, WFOM, rcatBias, rhid2Layer, rhid2Bias,
                           routLayer, routBias)
    in_maps = prepare_core_inputs(fwd, bwd, gold_heads, gold_rels, weights,
                                  routBias)
    res = run_bass_kernel_spmd(nc, in_maps, list(range(NCORES)))
    return assemble_output(res.results)


# revision 23
# speedup vs baseline: 1.0043x; 1.0043x over previous
"""Trainium2 Bass kernel for nn_ConcatRelationModule (gnn_message_passing).

Strategy: data-parallel over the edge dimension E across 8 NeuronCores.
 - Sharding/prep (host, untimed): edges split contiguously, 32768 per core.
   The per-edge head rows fwd[gold_heads] are materialized during sharding
   (the "gathered for the head indices" option of the sharding hint) and the
   modifier rows bwd[e+1] are a contiguous slice; both ship pre-transposed
   as fp16 [128, 32768] per core. The hinge mask ships as
   m2[p, b, r] = routBias[r] - 1024*(gold_rels[e]==r)  (e = b*128+p), fp16.
 - Device kernel, software-pipelined over 32 groups of 1024 edges:
     PE:     fov/mov (4+4 matmuls, n=512), h2 (4 matmuls) one group behind,
             scores edge-major (8 matmuls [128e,64l]) three groups behind so
             the scalar engine has a full step of slack to produce tanh(h2).
     Scalar: three tanh activations per group (the throughput floor,
             1 elem/lane/cycle @ 1.2 GHz).
     Vector: hinge = TT add(scores, m2) -> reduce_max (best wrong+bias)
             and reduce_min (gold+bias-1024, exact), 3 ops per group.
     DMA:    split across the SP hwdge queue and gpsimd swdge queue; one
             queue (~224 GB/s) cannot feed the ~25 MB/core input stream.
   lerrs accumulate in SBUF [128, 256] and are transposed out per half.
"""
import sys

sys.path.insert(0, "/opt/trn_rl_repo")

import numpy as np

import concourse.bass as bass
import concourse.bacc as bacc
import concourse.mybir as mybir
import concourse.tile as tile
from concourse.bass_utils import run_bass_kernel_spmd
from concourse.masks import make_identity

F32 = mybir.dt.float32
F16 = mybir.dt.float16

N = 262144
L = 128
H = 128
H2 = 128
R = 64
E = N - 1
NCORES = 8
EPC = N // NCORES            # edges per core (the very last edge is padding)
NB = EPC // 128              # 256 blocks of 128 edges
GE = 1024                    # edges per pipeline group
NG = EPC // GE               # 32 groups
BPG = GE // 128              # 8 blocks per group
MOFF = -1024.0               # gold-mask offset (exact in fp16/fp32)


def build_kernel():
    nc = bacc.Bacc("TRN2", target_bir_lowering=False, debug=False)

    # xin rows 0:128 = fwd[heads].T, 128:256 = bwd[mods].T  (fp16)
    xin_d = nc.declare_dram_parameter("xin", [2 * L, EPC], F16, isOutput=False)
    # m2[p, b*64+r] = routBias[r] + MOFF*(rels[b*128+p]==r)  (fp32: the
    # routBias term must survive next to the 1024 offset; fp16 ulp(1024)=1)
    m2_d = nc.declare_dram_parameter("m2", [128, NB * R], F32, isOutput=False)
    # wpk cols (partition = contraction dim k): 0:128 WFOH[0:128],
    # 128:256 WFOH[128:256], 256:384 WFOM[0:128], 384:512 WFOM[128:256],
    # 512:640 rhid2[0:128], 640:768 rhid2[128:256], 768:832 rout
    wpk_d = nc.declare_dram_parameter("wpk", [128, 832], F16, isOutput=False)
    # b3 cols: 0 bcat[:128], 1 bcat[128:], 2 rhid2Bias
    b3_d = nc.declare_dram_parameter("b3", [128, 3], F32, isOutput=False)

    lerr_d = nc.declare_dram_parameter("lerr", [EPC], F32, isOutput=True)
    lerr_v = lerr_d[:].rearrange("(b p) -> b p", p=128)

    with tile.TileContext(nc) as tc:
        with (
            tc.tile_pool(name="const", bufs=1) as cp,
            tc.tile_pool(name="inp", bufs=3) as gp,
            tc.tile_pool(name="msk", bufs=4) as mp,
            tc.tile_pool(name="act", bufs=2) as ap,
            tc.tile_pool(name="work", bufs=2) as wp,
            tc.tile_pool(name="psA", bufs=1, space="PSUM") as pa,
            tc.tile_pool(name="psB", bufs=1, space="PSUM") as pb,
            tc.tile_pool(name="psS", bufs=2, space="PSUM") as ps,
        ):
            # ---- constants ----
            ident = cp.tile([128, 128], F32, tag="ident")
            make_identity(nc, ident[:])

            wall = cp.tile([128, 832], F16, tag="wall")
            b3 = cp.tile([128, 3], F32, tag="b3")
            wfoh_f = wall[:, 0:128]
            wfoh_b = wall[:, 128:256]
            wfom_f = wall[:, 256:384]
            wfom_b = wall[:, 384:512]
            rh2_a = wall[:, 512:640]
            rh2_b = wall[:, 640:768]
            rout = wall[:, 768:832]
            bias_h = b3[:, 0:1]
            bias_m = b3[:, 1:2]
            bias_2 = b3[:, 2:3]

            goldm_acc = cp.tile([128, NB], F32, tag="goldm_acc")
            wrong_acc = cp.tile([128, NB], F32, tag="wrong_acc")

            xf = [None] * NG
            xb = [None] * NG
            m2t = [None] * NG
            h1 = [None] * NG
            h1m = [None] * NG
            h2s = [None] * NG

            def dma_x(s):
                # split issue across the SP hwdge queue and the gpsimd swdge
                # queue: a single queue (~224 GB/s) cannot feed the pipeline
                xf[s] = gp.tile([128, GE], F16, tag="xf", name=f"xf{s}")
                nc.sync.dma_start(out=xf[s][:], in_=xin_d[0:128, s * GE:(s + 1) * GE])
                xb[s] = gp.tile([128, GE], F16, tag="xb", name=f"xb{s}")
                nc.gpsimd.dma_start(out=xb[s][:], in_=xin_d[128:256, s * GE:(s + 1) * GE])

            def dma_m2(s):
                m2t[s] = mp.tile([128, BPG * R], F32, tag="m2t", name=f"m2t{s}", bufs=6)
                eng = nc.gpsimd if s % 2 else nc.sync
                eng.dma_start(
                    out=m2t[s][:], in_=m2_d[:, s * BPG * R:(s + 1) * BPG * R])

            def dma_group(s):
                dma_x(s)
                dma_m2(s)

            def phase_a(s):
                # fov/mov for group s + the two tanh h1 activations
                fovp = pa.tile([128, GE], F32, tag="fovp")
                nc.tensor.matmul(out=fovp[:, 0:512], lhsT=wfoh_f[:],
                                 rhs=xf[s][:, 0:512], start=True, stop=False)
                nc.tensor.matmul(out=fovp[:, 512:1024], lhsT=wfoh_f[:],
                                 rhs=xf[s][:, 512:1024], start=True, stop=False)
                nc.tensor.matmul(out=fovp[:, 0:512], lhsT=wfoh_b[:],
                                 rhs=xb[s][:, 0:512], start=False, stop=True)
                nc.tensor.matmul(out=fovp[:, 512:1024], lhsT=wfoh_b[:],
                                 rhs=xb[s][:, 512:1024], start=False, stop=True)
                movp = pb.tile([128, GE], F32, tag="movp")
                nc.tensor.matmul(out=movp[:, 0:512], lhsT=wfom_f[:],
                                 rhs=xf[s][:, 0:512], start=True, stop=False)
                nc.tensor.matmul(out=movp[:, 512:1024], lhsT=wfom_f[:],
                                 rhs=xf[s][:, 512:1024], start=True, stop=False)
                nc.tensor.matmul(out=movp[:, 0:512], lhsT=wfom_b[:],
                                 rhs=xb[s][:, 0:512], start=False, stop=True)
                nc.tensor.matmul(out=movp[:, 512:1024], lhsT=wfom_b[:],
                                 rhs=xb[s][:, 512:1024], start=False, stop=True)
                h1[s] = ap.tile([128, GE], F16, tag="h1", name=f"h1_{s}")
                nc.scalar.activation(
                    out=h1[s][:], in_=fovp[:],
                    func=mybir.ActivationFunctionType.Tanh, bias=bias_h[:, 0:1])
                h1m[s] = ap.tile([128, GE], F16, tag="h1m", name=f"h1m_{s}")
                nc.scalar.activation(
                    out=h1m[s][:], in_=movp[:],
                    func=mybir.ActivationFunctionType.Tanh, bias=bias_m[:, 0:1])

            def phase_b(s):
                # h2 for group s (one group behind phase_a)
                h2pp = pa.tile([128, GE], F32, tag="h2pp")
                nc.tensor.matmul(out=h2pp[:, 0:512], lhsT=rh2_a[:],
                                 rhs=h1[s][:, 0:512], start=True, stop=False)
                nc.tensor.matmul(out=h2pp[:, 512:1024], lhsT=rh2_a[:],
                                 rhs=h1[s][:, 512:1024], start=True, stop=False)
                nc.tensor.matmul(out=h2pp[:, 0:512], lhsT=rh2_b[:],
                                 rhs=h1m[s][:, 0:512], start=False, stop=True)
                nc.tensor.matmul(out=h2pp[:, 512:1024], lhsT=rh2_b[:],
                                 rhs=h1m[s][:, 512:1024], start=False, stop=True)
                h2s[s] = ap.tile([128, GE], F16, tag="h2s", name=f"h2s_{s}", bufs=3)
                nc.scalar.activation(
                    out=h2s[s][:], in_=h2pp[:],
                    func=mybir.ActivationFunctionType.Tanh, bias=bias_2[:, 0:1])
                h1[s] = h1m[s] = None

            def phase_c(s):
                # scores (edge-major) + hinge for group s (two groups behind)
                scp = ps.tile([128, BPG * R], F32, tag="scp")
                for j in range(BPG):
                    nc.tensor.matmul(
                        out=scp[:, j * R:(j + 1) * R],
                        lhsT=h2s[s][:, j * 128:(j + 1) * 128],
                        rhs=rout[:], start=True, stop=True)
                wm = wp.tile([128, BPG * R], F32, tag="wm")
                nc.vector.tensor_tensor(
                    out=wm[:], in0=scp[:], in1=m2t[s][:], op=mybir.AluOpType.add)
                nc.vector.reduce_max(
                    out=wrong_acc[:, s * BPG:(s + 1) * BPG],
                    in_=wm[:].rearrange("p (j r) -> p j r", r=R),
                    axis=mybir.AxisListType.X)
                nc.vector.tensor_reduce(
                    out=goldm_acc[:, s * BPG:(s + 1) * BPG],
                    in_=wm[:].rearrange("p (j r) -> p j r", r=R),
                    axis=mybir.AxisListType.X, op=mybir.AluOpType.min)
                h2s[s] = None
                m2t[s] = None
                xf[s] = xb[s] = None

            def emit_tail(half):
                # hinge + writeback for blocks [half*128, half*128+128)
                a = half * 128
                # gold = goldm - MOFF (exact); lerr = (wrong-gold > -1) ? wrong-gold : 0
                gold = wp.tile([128, 128], F32, tag="gold", name=f"gold{half}")
                nc.vector.tensor_scalar_add(
                    out=gold[:], in0=goldm_acc[:, a:a + 128], scalar1=-MOFF)
                dacc = wp.tile([128, 128], F32, tag="dacc", name=f"dacc{half}")
                nc.vector.tensor_tensor(
                    out=dacc[:], in0=wrong_acc[:, a:a + 128], in1=gold[:],
                    op=mybir.AluOpType.subtract)
                lerr_acc = wp.tile([128, 128], F32, tag="lerr_acc",
                                   name=f"lerr_acc{half}")
                nc.vector.scalar_tensor_tensor(
                    out=lerr_acc[:], in0=dacc[:], scalar=-1.0, in1=dacc[:],
                    op0=mybir.AluOpType.is_gt, op1=mybir.AluOpType.mult)
                otp = ps.tile([128, BPG * R], F32, tag="scp", name=f"otp{half}")
                nc.tensor.transpose(
                    out=otp[:, 0:128], in_=lerr_acc[:], identity=ident[:])
                osb = wp.tile([128, 128], F32, tag="osb", name=f"osb{half}")
                nc.scalar.copy(out=osb[:], in_=otp[:, 0:128])
                nc.sync.dma_start(out=lerr_v[a:a + 128, :], in_=osb[:])

            # ---- software-pipelined main loop ----
            # per step s: scores+hinge for s-3, fov/mov for s, h2 for s-1.
            # The 3-step trail gives the scalar engine a full step of slack
            # to finish tanh(h2) before the PE needs it for scores.
            # Startup order: group-0/1 x-data and the packed weights first so
            # the first matmul is not stuck behind a queue of small DMAs.
            # group-0 first halves, then weights, then the rest: the first
            # fov matmul needs only xf0[:, 0:512] + wall
            xf[0] = gp.tile([128, GE], F16, tag="xf", name="xf0")
            xb[0] = gp.tile([128, GE], F16, tag="xb", name="xb0")
            nc.sync.dma_start(out=xf[0][:, 0:512], in_=xin_d[0:128, 0:512])
            nc.gpsimd.dma_start(out=xb[0][:, 0:512], in_=xin_d[128:256, 0:512])
            nc.sync.dma_start(out=wall[:], in_=wpk_d[:])
            nc.gpsimd.dma_start(out=b3[:], in_=b3_d[:])
            nc.sync.dma_start(out=xf[0][:, 512:1024], in_=xin_d[0:128, 512:1024])
            nc.gpsimd.dma_start(out=xb[0][:, 512:1024], in_=xin_d[128:256, 512:1024])
            dma_x(1)
            dma_m2(0)
            dma_m2(1)
            for s in range(NG + 1):
                if s >= 3:
                    phase_c(s - 3)
                if s + 2 < NG:
                    dma_group(s + 2)
                if s < NG:
                    phase_a(s)
                if 1 <= s <= NG:
                    phase_b(s - 1)
                if s == NG:            # drain: C(NG-3) ran above; pull in C(NG-2)
                    phase_c(NG - 2)
                if s - 3 == NG // 2 - 1:
                    emit_tail(0)       # first half of blocks is complete
            phase_c(NG - 1)
            emit_tail(1)

    nc.compile()
    return nc


_NC_CACHE = {}


def _get_nc():
    if "nc" not in _NC_CACHE:
        _NC_CACHE["nc"] = build_kernel()
    return _NC_CACHE["nc"]


def make_weights(WFOH, WFOM, rcatBias, rhid2Layer, rhid2Bias, routLayer, routBias):
    WFOH = np.asarray(WFOH, np.float16)
    WFOM = np.asarray(WFOM, np.float16)
    rhid2 = np.asarray(rhid2Layer, np.float16)
    wpk = np.ascontiguousarray(np.hstack([
        WFOH[0:128], WFOH[128:256], WFOM[0:128], WFOM[128:256],
        rhid2[0:128], rhid2[128:256], np.asarray(routLayer, np.float16),
    ]))                                                 # [128, 832]
    bcat = np.asarray(rcatBias, np.float32).reshape(-1)
    b3 = np.ascontiguousarray(np.stack(
        [bcat[0:128], bcat[128:256],
         np.asarray(rhid2Bias, np.float32).reshape(-1)], axis=1))   # [128, 3]
    return dict(wpk=wpk, b3=b3)


def prepare_core_inputs(fwd, bwd, gold_heads, gold_rels, weights, routBias):
    """Shard edges contiguously; gather head rows / slice modifier rows.

    Core c owns edges [c*EPC, (c+1)*EPC); the single extra edge at the end
    (global index E) is padding with zero inputs.
    """
    fwd16 = np.asarray(fwd, dtype=np.float16)
    bwd16 = np.asarray(bwd, dtype=np.float16)
    heads = np.asarray(gold_heads, dtype=np.int64)
    rels = np.asarray(gold_rels, dtype=np.int64)
    rb32 = np.asarray(routBias, np.float32).reshape(-1)    # [R]

    in_maps = []
    for c in range(NCORES):
        lo, hi = c * EPC, (c + 1) * EPC
        if hi <= E:
            h_c = heads[lo:hi]
            fwd_rows = fwd16[h_c]                     # [EPC, L]
            bwd_rows = bwd16[lo + 1:hi + 1]           # contiguous modifiers
            r_c = rels[lo:hi]
        else:                                         # last core: pad 1 edge
            h_c = heads[lo:E]
            fwd_rows = np.zeros((EPC, L), np.float16)
            fwd_rows[:E - lo] = fwd16[h_c]
            bwd_rows = np.zeros((EPC, L), np.float16)
            bwd_rows[:E - lo] = bwd16[lo + 1:E + 1]
            r_c = np.zeros(EPC, np.int64)
            r_c[:E - lo] = rels[lo:E]

        xin = np.empty((2 * L, EPC), np.float16)
        xin[0:L] = fwd_rows.T
        xin[L:2 * L] = bwd_rows.T

        # m2[p, b, r] = routBias[r] + MOFF * (rels[b*128+p] == r)
        m2 = np.broadcast_to(rb32[None, None, :], (128, NB, R)).astype(np.float32).copy()
        idx = r_c.reshape(NB, 128).T                  # [128, NB]
        m2[np.arange(128)[:, None], np.arange(NB)[None, :], idx] += np.float32(MOFF)
        m2 = m2.reshape(128, NB * R)

        in_maps.append(dict(xin=xin, m2=m2, **weights))
    return in_maps


def assemble_output(results):
    lerr_full = np.empty(NCORES * EPC, dtype=np.float32)
    for c in range(NCORES):
        lerr_full[c * EPC:(c + 1) * EPC] = np.asarray(
            results[c]["lerr"], dtype=np.float32)
    return lerr_full[:E]


def kernel(fwd, bwd, gold_heads, gold_rels, WFOH, WFOM, rhidBias, rcatBias,
           rhid2Layer, rhid2Bias, routLayer, routBias):
    nc = _get_nc()
    weights = make_weights(WFOH, WFOM, rcatBias, rhid2Layer, rhid2Bias,
                           routLayer, routBias)
    in_maps = prepare_core_inputs(fwd, bwd, gold_heads, gold_rels, weights,
                                  routBias)
    res = run_bass_kernel_spmd(nc, in_maps, list(range(NCORES)))
    return assemble_output(res.results)


# revision 26
# speedup vs baseline: 1.0197x; 1.0153x over previous
"""Trainium2 Bass kernel for nn_ConcatRelationModule (gnn_message_passing).

Strategy: data-parallel over the edge dimension E across 8 NeuronCores.
 - Sharding/prep (host, untimed): edges split contiguously, 32768 per core.
   The per-edge head rows fwd[gold_heads] are materialized during sharding
   (the "gathered for the head indices" option of the sharding hint) and the
   modifier rows bwd[e+1] are a contiguous slice; both ship pre-transposed
   as fp16 [128, 32768] per core. The hinge mask ships as
   m2[p, b, r] = routBias[r] - 1024*(gold_rels[e]==r)  (e = b*128+p), fp16.
 - Device kernel, software-pipelined over 32 groups of 1024 edges:
     PE:     fov/mov (4+4 matmuls, n=512), h2 (4 matmuls) one group behind,
             scores edge-major (8 matmuls [128e,64l]) three groups behind so
             the scalar engine has a full step of slack to produce tanh(h2).
     Scalar: three tanh activations per group (the throughput floor,
             1 elem/lane/cycle @ 1.2 GHz).
     Vector: hinge = TT add(scores, m2) -> reduce_max (best wrong+bias)
             and reduce_min (gold+bias-1024, exact), 3 ops per group.
     DMA:    split across the SP hwdge queue and gpsimd swdge queue; one
             queue (~224 GB/s) cannot feed the ~25 MB/core input stream.
   lerrs accumulate in SBUF [128, 256] and are transposed out per half.
"""
import sys

sys.path.insert(0, "/opt/trn_rl_repo")

import numpy as np

import concourse.bass as bass
import concourse.bacc as bacc
import concourse.mybir as mybir
import concourse.tile as tile
from concourse.bass_utils import run_bass_kernel_spmd
from concourse.masks import make_identity

F32 = mybir.dt.float32
F16 = mybir.dt.float16

N = 262144
L = 128
H = 128
H2 = 128
R = 64
E = N - 1
NCORES = 8
EPC = N // NCORES            # edges per core (the very last edge is padding)
NB = EPC // 128              # 256 blocks of 128 edges
GE = 1024                    # edges per pipeline group
NG = EPC // GE               # 32 groups
BPG = GE // 128              # 8 blocks per group
MOFF = -1024.0               # gold-mask offset (exact in fp16/fp32)


def build_kernel():
    nc = bacc.Bacc("TRN2", target_bir_lowering=False, debug=False)

    # xin rows 0:128 = fwd[heads].T, 128:256 = bwd[mods].T  (fp16)
    xin_d = nc.declare_dram_parameter("xin", [2 * L, EPC], F16, isOutput=False)
    # m2[p, b*64+r] = routBias[r] + MOFF*(rels[b*128+p]==r)  (fp32: the
    # routBias term must survive next to the 1024 offset; fp16 ulp(1024)=1)
    m2_d = nc.declare_dram_parameter("m2", [128, NB * R], F32, isOutput=False)
    # wpk cols (partition = contraction dim k): 0:128 WFOH[0:128],
    # 128:256 WFOH[128:256], 256:384 WFOM[0:128], 384:512 WFOM[128:256],
    # 512:640 rhid2[0:128], 640:768 rhid2[128:256], 768:832 rout
    wpk_d = nc.declare_dram_parameter("wpk", [128, 832], F16, isOutput=False)
    # b3 cols: 0 bcat[:128], 1 bcat[128:], 2 rhid2Bias
    b3_d = nc.declare_dram_parameter("b3", [128, 3], F32, isOutput=False)

    lerr_d = nc.declare_dram_parameter("lerr", [EPC], F32, isOutput=True)
    lerr_v = lerr_d[:].rearrange("(b p) -> b p", p=128)

    with tile.TileContext(nc) as tc:
        with (
            tc.tile_pool(name="const", bufs=1) as cp,
            tc.tile_pool(name="inp", bufs=3) as gp,
            tc.tile_pool(name="msk", bufs=4) as mp,
            tc.tile_pool(name="act", bufs=2) as ap,
            tc.tile_pool(name="work", bufs=2) as wp,
            tc.tile_pool(name="psA", bufs=1, space="PSUM") as pa,
            tc.tile_pool(name="psB", bufs=1, space="PSUM") as pb,
            tc.tile_pool(name="psS", bufs=2, space="PSUM") as ps,
        ):
            # ---- constants ----
            ident = cp.tile([128, 128], F32, tag="ident")
            make_identity(nc, ident[:])

            # PE warm-up: keep the tensor engine busy through the initial
            # DMA window so its p-state clock is fully ramped (2.4 GHz needs
            # ~3us of continuous work) when the first fov matmul issues.
            warm = ps.tile([128, BPG * R], F32, tag="scp", name="warm")
            for _ in range(6):
                nc.tensor.transpose(
                    out=warm[:, 0:128], in_=ident[:], identity=ident[:])

            wall = cp.tile([128, 832], F16, tag="wall")
            b3 = cp.tile([128, 3], F32, tag="b3")
            wfoh_f = wall[:, 0:128]
            wfoh_b = wall[:, 128:256]
            wfom_f = wall[:, 256:384]
            wfom_b = wall[:, 384:512]
            rh2_a = wall[:, 512:640]
            rh2_b = wall[:, 640:768]
            rout = wall[:, 768:832]
            bias_h = b3[:, 0:1]
            bias_m = b3[:, 1:2]
            bias_2 = b3[:, 2:3]

            goldm_acc = cp.tile([128, NB], F32, tag="goldm_acc")
            wrong_acc = cp.tile([128, NB], F32, tag="wrong_acc")

            xf = [None] * NG
            xb = [None] * NG
            m2t = [None] * NG
            h1 = [None] * NG
            h1m = [None] * NG
            h2s = [None] * NG

            def dma_x(s):
                # split issue across the two hwdge queues (SP + Act): a single
                # queue (~224 GB/s) cannot feed the ~25 MB/core input stream
                xf[s] = gp.tile([128, GE], F16, tag="xf", name=f"xf{s}")
                nc.sync.dma_start(out=xf[s][:], in_=xin_d[0:128, s * GE:(s + 1) * GE])
                xb[s] = gp.tile([128, GE], F16, tag="xb", name=f"xb{s}")
                nc.scalar.dma_start(out=xb[s][:], in_=xin_d[128:256, s * GE:(s + 1) * GE])

            def dma_m2(s):
                m2t[s] = mp.tile([128, BPG * R], F32, tag="m2t", name=f"m2t{s}", bufs=6)
                eng = nc.scalar if s % 2 else nc.sync
                eng.dma_start(
                    out=m2t[s][:], in_=m2_d[:, s * BPG * R:(s + 1) * BPG * R])

            def dma_group(s):
                dma_x(s)
                dma_m2(s)

            def phase_a(s):
                # fov/mov for group s + the two tanh h1 activations
                fovp = pa.tile([128, GE], F32, tag="fovp")
                nc.tensor.matmul(out=fovp[:, 0:512], lhsT=wfoh_f[:],
                                 rhs=xf[s][:, 0:512], start=True, stop=False)
                nc.tensor.matmul(out=fovp[:, 512:1024], lhsT=wfoh_f[:],
                                 rhs=xf[s][:, 512:1024], start=True, stop=False)
                nc.tensor.matmul(out=fovp[:, 0:512], lhsT=wfoh_b[:],
                                 rhs=xb[s][:, 0:512], start=False, stop=True)
                nc.tensor.matmul(out=fovp[:, 512:1024], lhsT=wfoh_b[:],
                                 rhs=xb[s][:, 512:1024], start=False, stop=True)
                movp = pb.tile([128, GE], F32, tag="movp")
                nc.tensor.matmul(out=movp[:, 0:512], lhsT=wfom_f[:],
                                 rhs=xf[s][:, 0:512], start=True, stop=False)
                nc.tensor.matmul(out=movp[:, 512:1024], lhsT=wfom_f[:],
                                 rhs=xf[s][:, 512:1024], start=True, stop=False)
                nc.tensor.matmul(out=movp[:, 0:512], lhsT=wfom_b[:],
                                 rhs=xb[s][:, 0:512], start=False, stop=True)
                nc.tensor.matmul(out=movp[:, 512:1024], lhsT=wfom_b[:],
                                 rhs=xb[s][:, 512:1024], start=False, stop=True)
                h1[s] = ap.tile([128, GE], F16, tag="h1", name=f"h1_{s}")
                nc.scalar.activation(
                    out=h1[s][:], in_=fovp[:],
                    func=mybir.ActivationFunctionType.Tanh, bias=bias_h[:, 0:1])
                h1m[s] = ap.tile([128, GE], F16, tag="h1m", name=f"h1m_{s}")
                nc.scalar.activation(
                    out=h1m[s][:], in_=movp[:],
                    func=mybir.ActivationFunctionType.Tanh, bias=bias_m[:, 0:1])

            def phase_b(s):
                # h2 for group s (one group behind phase_a)
                h2pp = pa.tile([128, GE], F32, tag="h2pp")
                nc.tensor.matmul(out=h2pp[:, 0:512], lhsT=rh2_a[:],
                                 rhs=h1[s][:, 0:512], start=True, stop=False)
                nc.tensor.matmul(out=h2pp[:, 512:1024], lhsT=rh2_a[:],
                                 rhs=h1[s][:, 512:1024], start=True, stop=False)
                nc.tensor.matmul(out=h2pp[:, 0:512], lhsT=rh2_b[:],
                                 rhs=h1m[s][:, 0:512], start=False, stop=True)
                nc.tensor.matmul(out=h2pp[:, 512:1024], lhsT=rh2_b[:],
                                 rhs=h1m[s][:, 512:1024], start=False, stop=True)
                h2s[s] = ap.tile([128, GE], F16, tag="h2s", name=f"h2s_{s}", bufs=3)
                nc.scalar.activation(
                    out=h2s[s][:], in_=h2pp[:],
                    func=mybir.ActivationFunctionType.Tanh, bias=bias_2[:, 0:1])
                h1[s] = h1m[s] = None

            def phase_c(s):
                # scores (edge-major) + hinge for group s (two groups behind)
                scp = ps.tile([128, BPG * R], F32, tag="scp")
                for j in range(BPG):
                    nc.tensor.matmul(
                        out=scp[:, j * R:(j + 1) * R],
                        lhsT=h2s[s][:, j * 128:(j + 1) * 128],
                        rhs=rout[:], start=True, stop=True)
                wm = wp.tile([128, BPG * R], F32, tag="wm")
                nc.vector.tensor_tensor(
                    out=wm[:], in0=scp[:], in1=m2t[s][:], op=mybir.AluOpType.add)
                nc.vector.reduce_max(
                    out=wrong_acc[:, s * BPG:(s + 1) * BPG],
                    in_=wm[:].rearrange("p (j r) -> p j r", r=R),
                    axis=mybir.AxisListType.X)
                nc.vector.tensor_reduce(
                    out=goldm_acc[:, s * BPG:(s + 1) * BPG],
                    in_=wm[:].rearrange("p (j r) -> p j r", r=R),
                    axis=mybir.AxisListType.X, op=mybir.AluOpType.min)
                h2s[s] = None
                m2t[s] = None
                xf[s] = xb[s] = None

            def emit_tail(half):
                # hinge + writeback for blocks [half*128, half*128+128)
                a = half * 128
                # gold = goldm - MOFF (exact); lerr = (wrong-gold > -1) ? wrong-gold : 0
                gold = wp.tile([128, 128], F32, tag="gold", name=f"gold{half}")
                nc.vector.tensor_scalar_add(
                    out=gold[:], in0=goldm_acc[:, a:a + 128], scalar1=-MOFF)
                dacc = wp.tile([128, 128], F32, tag="dacc", name=f"dacc{half}")
                nc.vector.tensor_tensor(
                    out=dacc[:], in0=wrong_acc[:, a:a + 128], in1=gold[:],
                    op=mybir.AluOpType.subtract)
                lerr_acc = wp.tile([128, 128], F32, tag="lerr_acc",
                                   name=f"lerr_acc{half}")
                nc.vector.scalar_tensor_tensor(
                    out=lerr_acc[:], in0=dacc[:], scalar=-1.0, in1=dacc[:],
                    op0=mybir.AluOpType.is_gt, op1=mybir.AluOpType.mult)
                otp = ps.tile([128, BPG * R], F32, tag="scp", name=f"otp{half}")
                nc.tensor.transpose(
                    out=otp[:, 0:128], in_=lerr_acc[:], identity=ident[:])
                osb = wp.tile([128, 128], F32, tag="osb", name=f"osb{half}")
                nc.scalar.copy(out=osb[:], in_=otp[:, 0:128])
                nc.sync.dma_start(out=lerr_v[a:a + 128, :], in_=osb[:])

            # ---- software-pipelined main loop ----
            # per step s: scores+hinge for s-3, fov/mov for s, h2 for s-1.
            # The 3-step trail gives the scalar engine a full step of slack
            # to finish tanh(h2) before the PE needs it for scores.
            # Startup order: group-0/1 x-data and the packed weights first so
            # the first matmul is not stuck behind a queue of small DMAs.
            dma_x(0)
            nc.sync.dma_start(out=wall[:], in_=wpk_d[:])
            nc.scalar.dma_start(out=b3[:], in_=b3_d[:])
            dma_x(1)
            dma_m2(0)
            dma_m2(1)
            for s in range(NG + 1):
                if s >= 3:
                    phase_c(s - 3)
                if s + 2 < NG:
                    dma_group(s + 2)
                if s < NG:
                    phase_a(s)
                if 1 <= s <= NG:
                    phase_b(s - 1)
                if s == NG:            # drain: C(NG-3) ran above; pull in C(NG-2)
                    phase_c(NG - 2)
                if s - 3 == NG // 2 - 1:
                    emit_tail(0)       # first half of blocks is complete
            phase_c(NG - 1)
            emit_tail(1)

    nc.compile()
    return nc


_NC_CACHE = {}


def _get_nc():
    if "nc" not in _NC_CACHE:
        _NC_CACHE["nc"] = build_kernel()
    return _NC_CACHE["nc"]


def make_weights(WFOH, WFOM, rcatBias, rhid2Layer, rhid2Bias, routLayer, routBias):
    WFOH = np.asarray(WFOH, np.float16)
    WFOM = np.asarray(WFOM, np.float16)
    rhid2 = np.asarray(rhid2Layer, np.float16)
    wpk = np.ascontiguousarray(np.hstack([
        WFOH[0:128], WFOH[128:256], WFOM[0:128], WFOM[128:256],
        rhid2[0:128], rhid2[128:256], np.asarray(routLayer, np.float16),
    ]))                                                 # [128, 832]
    bcat = np.asarray(rcatBias, np.float32).reshape(-1)
    b3 = np.ascontiguousarray(np.stack(
        [bcat[0:128], bcat[128:256],
         np.asarray(rhid2Bias, np.float32).reshape(-1)], axis=1))   # [128, 3]
    return dict(wpk=wpk, b3=b3)


def prepare_core_inputs(fwd, bwd, gold_heads, gold_rels, weights, routBias):
    """Shard edges contiguously; gather head rows / slice modifier rows.

    Core c owns edges [c*EPC, (c+1)*EPC); the single extra edge at the end
    (global index E) is padding with zero inputs.
    """
    fwd16 = np.asarray(fwd, dtype=np.float16)
    bwd16 = np.asarray(bwd, dtype=np.float16)
    heads = np.asarray(gold_heads, dtype=np.int64)
    rels = np.asarray(gold_rels, dtype=np.int64)
    rb32 = np.asarray(routBias, np.float32).reshape(-1)    # [R]

    in_maps = []
    for c in range(NCORES):
        lo, hi = c * EPC, (c + 1) * EPC
        if hi <= E:
            h_c = heads[lo:hi]
            fwd_rows = fwd16[h_c]                     # [EPC, L]
            bwd_rows = bwd16[lo + 1:hi + 1]           # contiguous modifiers
            r_c = rels[lo:hi]
        else:                                         # last core: pad 1 edge
            h_c = heads[lo:E]
            fwd_rows = np.zeros((EPC, L), np.float16)
            fwd_rows[:E - lo] = fwd16[h_c]
            bwd_rows = np.zeros((EPC, L), np.float16)
            bwd_rows[:E - lo] = bwd16[lo + 1:E + 1]
            r_c = np.zeros(EPC, np.int64)
            r_c[:E - lo] = rels[lo:E]

        xin = np.empty((2 * L, EPC), np.float16)
        xin[0:L] = fwd_rows.T
        xin[L:2 * L] = bwd_rows.T

        # m2[p, b, r] = routBias[r] + MOFF * (rels[b*128+p] == r)
        m2 = np.broadcast_to(rb32[None, None, :], (128, NB, R)).astype(np.float32).copy()
        idx = r_c.reshape(NB, 128).T                  # [128, NB]
        m2[np.arange(128)[:, None], np.arange(NB)[None, :], idx] += np.float32(MOFF)
        m2 = m2.reshape(128, NB * R)

        in_maps.append(dict(xin=xin, m2=m2, **weights))
    return in_maps


def assemble_output(results):
    lerr_full = np.empty(NCORES * EPC, dtype=np.float32)
    for c in range(NCORES):
        lerr_full[c * EPC:(c + 1) * EPC] = np.asarray(
            results[c]["lerr"], dtype=np.float32)
    return lerr_full[:E]


def kernel(fwd, bwd, gold_heads, gold_rels, WFOH, WFOM, rhidBias, rcatBias,
           rhid2Layer, rhid2Bias, routLayer, routBias):
    nc = _get_nc()
    weights = make_weights(WFOH, WFOM, rcatBias, rhid2Layer, rhid2Bias,
                           routLayer, routBias)
    in_maps = prepare_core_inputs(fwd, bwd, gold_heads, gold_rels, weights,
                                  routBias)
    res = run_bass_kernel_spmd(nc, in_maps, list(range(NCORES)))
    return assemble_output(res.results)
